# revision 15
# baseline (speedup 1.0000x reference)
"""GRU layer kernel for Trainium2 (8 NeuronCores, batch-data-parallel).

x: [256, 128, 2048] f32, W/U: [128, 384], b: [384] -> y: [256, 128, 2048] f32

The end-to-end wall time of kernel() is dominated by the axon tunnel
(~40MB/s up, ~45MB/s down, full-duplex), so the design minimizes and
pipelines host<->device traffic:
  - inputs cross the tunnel as f16 (or fp8) in their NATURAL layout
    (no host-side transposes); the layout change to [D, t, s] happens
    on-device via strided-DMA access patterns (1KB+ contiguous lines).
  - the batch is split into NSLICES slices; uploads, device execution
    and downloads of different slices overlap (the tunnel is duplex).
  - the jitted dispatch function is cached across kernel() calls, and
    donated output buffers are recycled device-side so no zero-buffers
    cross the tunnel in steady state.

Device kernel (per core, per slice of BS sequences):
  x dram:  [BS, D, T] 16-bit, y dram: [BS, D, T] 16-bit
  SBUF x:  [D, BS, T] f16 (DMA transposes via access pattern)
  SBUF h:  [D, BS, T+1] f16 history, written per step, DMA'd out once
  PSUM gx: [D, 3, BS, WS] f32 per window (3 bulk W matmuls)
  PSUM gh: [D, 3, BS] f32 per step (3 recurrent U matmuls)
  per step: 3 MM + 2 ACT (sigmoid zr, tanh n) + 6 DVE ops.
PSUM discipline: first matmul of each PSUM tile uses start=True (clears
the bank's has_written bits); later matmuls into other regions of the
same tile use start=False (fresh regions overwrite). All matmul output
APs are contiguous (strided PSUM outs crash the device).
"""

import sys
import numpy as np
from collections import deque
from contextlib import ExitStack
from concurrent.futures import ThreadPoolExecutor

sys.path.insert(0, "/opt/trn_rl_repo")

B_TOT, D, T = 256, 128, 2048
NCORES = 8
B_CORE = B_TOT // NCORES          # 32 sequences per core

# --- tunables ---------------------------------------------------------
NSLICES = 4                       # pipeline depth over the batch axis
BS = B_CORE // NSLICES            # sequences per core per slice
WS = 16                           # recurrence steps per PSUM gx window
X_DT = "f16"                      # tunnel dtype for x: "f16" | "f8"
Y_DT = "i8"                       # tunnel dtype for y: "f16" | "i8"
Y_SCALE = 127.0                   # int8 output scale (|h| < 1 always)
DL_THREADS = 8                    # download worker threads
UL_THREADS = 8                    # upload worker threads

_prog_cache = {}


def _np_dt(name):
    import ml_dtypes
    return {"f16": np.float16, "f8": ml_dtypes.float8_e4m3fn,
            "i8": np.int8}[name]


def _build(b_nonzero: bool):
    import concourse.bacc as bacc
    import concourse.tile as tile
    import concourse.mybir as mybir

    F32 = mybir.dt.float32
    F16 = mybir.dt.float16
    XDT = {"f16": F16, "f8": mybir.dt.float8e4}[X_DT]
    YDT = {"f16": F16, "i8": mybir.dt.int8}[Y_DT]
    SIG = mybir.ActivationFunctionType.Sigmoid
    TANH = mybir.ActivationFunctionType.Tanh

    NW = T // WS                  # gx windows over the full sequence

    nc = bacc.Bacc("TRN2", target_bir_lowering=False, debug=False,
                   num_devices=NCORES)
    x_d = nc.declare_dram_parameter("x", [BS, D, T], XDT, isOutput=False)
    y_d = nc.declare_dram_parameter("y", [BS, D, T], YDT, isOutput=True)
    w_d = nc.declare_dram_parameter("w", [D, 3 * D], F16, isOutput=False)
    u_d = nc.declare_dram_parameter("u", [D, 3 * D], F16, isOutput=False)
    if b_nonzero:
        b_d = nc.declare_dram_parameter("b", [3, D], F32, isOutput=False)

    with tile.TileContext(nc) as tc:
        with ExitStack() as ctx:
            wpool = ctx.enter_context(tc.tile_pool(name="wts", bufs=1))
            xpool = ctx.enter_context(tc.tile_pool(name="xin", bufs=1))
            hpool = ctx.enter_context(tc.tile_pool(name="hh", bufs=1))
            spool = ctx.enter_context(tc.tile_pool(name="small", bufs=3))
            gxpool = ctx.enter_context(
                tc.tile_pool(name="psgx", bufs=2, space="PSUM"))

            wz = wpool.tile([D, D], F16, name="wz")
            wr = wpool.tile([D, D], F16, name="wr")
            wn = wpool.tile([D, D], F16, name="wn")
            uz = wpool.tile([D, D], F16, name="uz")
            ur = wpool.tile([D, D], F16, name="ur")
            un = wpool.tile([D, D], F16, name="un")
            for j, (wt, ut) in enumerate([(wz, uz), (wr, ur), (wn, un)]):
                nc.sync.dma_start(wt[:], w_d[:, j * D:(j + 1) * D])
                nc.sync.dma_start(ut[:], u_d[:, j * D:(j + 1) * D])
            if b_nonzero:
                bz = wpool.tile([D, 1], F32, name="bz")
                br = wpool.tile([D, 1], F32, name="br")
                bn = wpool.tile([D, 1], F32, name="bn")
                for j, bt in enumerate([bz, br, bn]):
                    nc.sync.dma_start(bt[:], b_d[j:j + 1, :].transpose([1, 0]))

            # x arrives batch-major (cheap DMA: contiguous t-lines), then
            # one DVE shuffle to the t-major compute layout [D, T, BS].
            x_raw = xpool.tile([D, BS, T], XDT, name="xraw")
            nc.sync.dma_start(x_raw[:], x_d[:].transpose([1, 0, 2]))
            x_sb = xpool.tile([D, T, BS], F16, name="x")
            nc.vector.tensor_copy(x_sb[:], x_raw[:].transpose([0, 2, 1]))

            hh = hpool.tile([D, T + 1, BS], F16, name="h")
            nc.vector.memset(hh[:, 0:1, :], 0.0)

            for w in range(NW):
                # window PSUM tile: q slots 0=z 1=r 2=xn 3=ghn. One
                # start=True per tile (first bulk matmul) clears the
                # bank; recurrent matmuls accumulate (q0/q1) or fill a
                # fresh region (q3). All matmul out APs contiguous.
                gx = gxpool.tile([D, 4, WS, BS], F32, tag="gx",
                                 name=f"gx{w}")
                xg = x_sb[:, w * WS:(w + 1) * WS, :]
                nc.tensor.matmul(gx[:, 0:1, :, :], wz[:], xg,
                                 start=True, stop=True,
                                 skip_group_check=True)
                nc.tensor.matmul(gx[:, 1:2, :, :], wr[:], xg,
                                 start=False, stop=True,
                                 skip_group_check=True)
                nc.tensor.matmul(gx[:, 2:3, :, :], wn[:], xg,
                                 start=False, stop=True,
                                 skip_group_check=True)

                for tl in range(WS):
                    t = w * WS + tl
                    h_at = hh[:, t:t + 1, :]
                    nc.tensor.matmul(gx[:, 0:1, tl:tl + 1, :], uz[:],
                                     h_at, start=False, stop=True,
                                     skip_group_check=True)
                    nc.tensor.matmul(gx[:, 1:2, tl:tl + 1, :], ur[:],
                                     h_at, start=False, stop=True,
                                     skip_group_check=True)
                    nc.tensor.matmul(gx[:, 3:4, tl:tl + 1, :], un[:],
                                     h_at, start=False, stop=True,
                                     skip_group_check=True)

                    zr = spool.tile([D, 2, BS], F32, tag="zr",
                                    name=f"zr{t}")
                    if b_nonzero:
                        nc.scalar.activation(zr[:, 0:1, :],
                                             gx[:, 0:1, tl:tl + 1, :],
                                             SIG, bias=bz[:])
                        nc.scalar.activation(zr[:, 1:2, :],
                                             gx[:, 1:2, tl:tl + 1, :],
                                             SIG, bias=br[:])
                    else:
                        nc.scalar.activation(zr[:],
                                             gx[:, 0:2, tl:tl + 1, :],
                                             SIG)

                    t1 = spool.tile([D, BS], F32, tag="t1", name=f"t1{t}")
                    nc.vector.tensor_mul(t1[:], zr[:, 1:2, :],
                                         gx[:, 3:4, tl:tl + 1, :])
                    t2 = spool.tile([D, BS], F32, tag="t2", name=f"t2{t}")
                    nc.vector.tensor_add(t2[:], t1[:],
                                         gx[:, 2:3, tl:tl + 1, :])
                    nt = spool.tile([D, BS], F32, tag="nt", name=f"nt{t}")
                    if b_nonzero:
                        nc.scalar.activation(nt[:], t2[:], TANH, bias=bn[:])
                    else:
                        nc.scalar.activation(nt[:], t2[:], TANH)

                    dd = spool.tile([D, BS], F32, tag="dd", name=f"dd{t}")
                    nc.vector.tensor_sub(dd[:], h_at, nt[:])
                    ee = spool.tile([D, BS], F32, tag="ee", name=f"ee{t}")
                    nc.vector.tensor_mul(ee[:], zr[:, 0:1, :], dd[:])
                    nc.vector.tensor_add(hh[:, t + 1:t + 2, :], ee[:],
                                         nt[:])

            # shuffle back to batch-major and DMA out (contiguous t-lines)
            y_stg = hpool.tile([D, BS, T], YDT, name="ystg")
            if Y_DT == "i8":
                nc.vector.tensor_scalar_mul(
                    y_stg[:], hh[:, 1:T + 1, :].transpose([0, 2, 1]),
                    Y_SCALE)
            else:
                nc.vector.tensor_copy(
                    y_stg[:], hh[:, 1:T + 1, :].transpose([0, 2, 1]))
            nc.sync.dma_start(y_d[:].transpose([1, 0, 2]), y_stg[:])
    nc.compile()
    return nc


class _Runner:
    """Cached jitted SPMD dispatch for one compiled bass program.

    Mirrors concourse.bass_utils.run_bass_kernel_spmd's axon branch
    (bass2jax._bass_exec_p under jit+shard_map on the 8 NeuronCores) but
    keeps the traced function and donated output buffers alive across
    calls so steady-state calls move only x up and y down the tunnel.
    """

    def __init__(self, nc):
        import jax
        import concourse.mybir as mybir
        from concourse import bass2jax
        from jax.sharding import Mesh, PartitionSpec, NamedSharding
        from jax.experimental.shard_map import shard_map

        bass2jax.install_neuronx_cc_hook()
        self.nc = nc
        part_name = (nc.partition_id_tensor.name
                     if nc.partition_id_tensor else None)
        in_names, out_names, out_avals = [], [], []
        for alloc in nc.m.functions[0].allocations:
            if not isinstance(alloc, mybir.MemoryLocationSet):
                continue
            name = alloc.memorylocations[0].name
            if alloc.kind == "ExternalInput":
                if name != part_name:
                    in_names.append(name)
            elif alloc.kind == "ExternalOutput":
                out_names.append(name)
                out_avals.append(jax.core.ShapedArray(
                    tuple(alloc.tensor_shape), mybir.dt.np(alloc.dtype)))
        self.in_names = in_names
        self.out_names = out_names
        all_names = in_names + out_names
        n_args = len(all_names)
        if part_name is not None:
            all_names = all_names + [part_name]
        all_names = tuple(all_names)

        def _body(*args):
            operands = list(args)
            if part_name is not None:
                operands.append(bass2jax.partition_id_tensor())
            outs = bass2jax._bass_exec_p.bind(
                *operands,
                out_avals=tuple(out_avals),
                in_names=all_names,
                out_names=tuple(out_names),
                lowering_input_output_aliases=(),
                sim_require_finite=True,
                sim_require_nnan=True,
                nc=nc,
            )
            return tuple(outs)

        devices = jax.devices()[:NCORES]
        self.mesh = Mesh(np.asarray(devices), ("core",))
        P = PartitionSpec
        self.sharding = NamedSharding(self.mesh, P("core"))
        donate = tuple(range(len(in_names), n_args))
        self.jitted = jax.jit(
            shard_map(_body, mesh=self.mesh, in_specs=(P("core"),) * n_args,
                      out_specs=(P("core"),) * len(out_names),
                      check_rep=False),
            donate_argnums=donate, keep_unused=True)
        self.donors = deque()     # device buffers reusable as donated outs
        self.out_shape = tuple(out_avals[0].shape)
        self.out_np_dtype = np.dtype(out_avals[0].dtype)

    def get_donor(self):
        import jax
        if self.donors:
            return self.donors.popleft()
        z = np.zeros((NCORES * self.out_shape[0],) + self.out_shape[1:],
                     self.out_np_dtype)
        return jax.device_put(z, self.sharding)


def _get_nc(b_nonzero: bool):
    key = ("nc", b_nonzero, NSLICES, X_DT, Y_DT)
    if key not in _prog_cache:
        _prog_cache[key] = _build(b_nonzero)
    return _prog_cache[key]


def _get_runner(b_nonzero: bool) -> "_Runner":
    key = ("run", b_nonzero, NSLICES, X_DT, Y_DT)
    if key not in _prog_cache:
        _prog_cache[key] = _Runner(_get_nc(b_nonzero))
    return _prog_cache[key]


def _kernel_fast(x, W, U, b, b_nonzero):
    import jax

    runner = _get_runner(b_nonzero)
    xdt = _np_dt(X_DT)

    wg = jax.device_put(np.tile(W.astype(np.float16), (NCORES, 1)),
                        runner.sharding)
    ug = jax.device_put(np.tile(U.astype(np.float16), (NCORES, 1)),
                        runner.sharding)
    extra = [wg, ug]
    if b_nonzero:
        bg = jax.device_put(
            np.tile(np.ascontiguousarray(b.reshape(3, D)), (NCORES, 1)),
            runner.sharding)
        extra.append(bg)

    y = np.empty((B_TOT, D, T), dtype=np.float32)
    inv_scale = np.float32(1.0 / Y_SCALE)

    def fetch_shard(data):
        # RPC fetch only (releases the GIL); dequant happens on the
        # main thread to avoid starving the upload casts
        return np.asarray(data)

    def store_shard(shard, core, sl):
        r0 = core * B_CORE + sl * BS
        if Y_DT == "i8":
            np.multiply(shard, inv_scale, out=y[r0:r0 + BS])
        else:
            y[r0:r0 + BS] = shard

    import os
    import time
    dbg = bool(os.environ.get("KERNEL_DEBUG_TIMING"))
    t_start = time.time()

    devices = runner.mesh.devices.flat
    glob_shape = (NCORES * BS, D, T)

    def cast_put_shard(sl, i):
        # cast + upload one per-core shard; numpy's cast releases the
        # GIL, and the tunnel is stream-limited (~20MB/s per stream,
        # ~50MB/s aggregate with >=4 concurrent streams), so running
        # all (slice, core) tasks on a pool keeps the pipe full
        src = x[i * B_CORE + sl * BS: i * B_CORE + (sl + 1) * BS]
        return jax.device_put(src.astype(xdt), devices[i])

    pending = []
    with ThreadPoolExecutor(DL_THREADS) as pool, \
            ThreadPoolExecutor(UL_THREADS) as upool:
        put_futs = {(sl, i): upool.submit(cast_put_shard, sl, i)
                    for sl in range(NSLICES) for i in range(NCORES)}
        for sl in range(NSLICES):
            t0 = time.time()
            x_dev = jax.make_array_from_single_device_arrays(
                glob_shape, runner.sharding,
                [put_futs[(sl, i)].result() for i in range(NCORES)])
            t1 = time.time()
            donor = runner.get_donor()
            (y_dev,) = runner.jitted(x_dev, *extra, donor)
            t2 = time.time()
            # pre-issue host copies so fetch RPCs pipeline instead of
            # paying a round-trip latency per shard
            datas = [s.data for s in y_dev.addressable_shards]
            for d_ in datas:
                d_.copy_to_host_async()
            futs = [pool.submit(fetch_shard, datas[i])
                    for i in range(NCORES)]
            pending.append((y_dev, futs))
            if dbg:
                print(f"  sl{sl}: cast {t1-t0:.2f} put+disp {t2-t1:.2f} "
                      f"(t={t2-t_start:.2f})", flush=True)
        for sl, (y_dev, futs) in enumerate(pending):
            for i, f in enumerate(futs):
                store_shard(f.result(), i, sl)
            if dbg:
                print(f"  sl{sl} stored at t={time.time()-t_start:.2f}",
                      flush=True)
            runner.donors.append(y_dev)
    if dbg:
        print(f"  kernel_fast total {time.time()-t_start:.2f}", flush=True)
    return y


def _kernel_fallback(x, W, U, b, b_nonzero):
    """Library-path fallback: same program via run_bass_kernel_spmd."""
    from concourse.bass_utils import run_bass_kernel_spmd

    nc = _get_nc(b_nonzero)
    xdt = _np_dt(X_DT)
    wg = {"w": W.astype(np.float16), "u": U.astype(np.float16)}
    if b_nonzero:
        wg["b"] = np.ascontiguousarray(b.reshape(3, D))
    y = np.empty((B_TOT, D, T), dtype=np.float32)
    for sl in range(NSLICES):
        in_maps = []
        for i in range(NCORES):
            src = x[i * B_CORE + sl * BS: i * B_CORE + (sl + 1) * BS]
            m = {"x": np.ascontiguousarray(src).astype(xdt)}
            m.update(wg)
            in_maps.append(m)
        res = run_bass_kernel_spmd(nc, in_maps, list(range(NCORES)))
        for i in range(NCORES):
            yi = res.results[i]["y"]
            r0 = i * B_CORE + sl * BS
            if Y_DT == "i8":
                y[r0:r0 + BS] = yi.astype(np.float32) * (1.0 / Y_SCALE)
            else:
                y[r0:r0 + BS] = yi
    return y


def kernel(x, W, U, b):
    x = np.asarray(x, dtype=np.float32)
    W = np.asarray(W, dtype=np.float32)
    U = np.asarray(U, dtype=np.float32)
    b = np.asarray(b, dtype=np.float32)
    b_nonzero = bool(np.any(b != 0.0))
    try:
        return _kernel_fast(x, W, U, b, b_nonzero)
    except Exception:
        import traceback
        traceback.print_exc()
        return _kernel_fallback(x, W, U, b, b_nonzero)


# revision 19
# speedup vs baseline: 1.4929x; 1.4929x over previous
"""GRU layer kernel for Trainium2 (8 NeuronCores, batch-data-parallel).

x: [256, 128, 2048] f32, W/U: [128, 384], b: [384] -> y: [256, 128, 2048] f32

The end-to-end wall time of kernel() is dominated by the axon tunnel
(~20MB/s per stream, ~50MB/s aggregate with >=4 concurrent streams)
and the 1-CPU client host, so the design minimizes and pipelines
host<->device traffic:
  - x crosses the tunnel as f16 (fp8 fails numerically: the gx noise
    drives an h random walk, rel err 0.17) in its NATURAL layout (no
    host-side transposes); y returns as int8 (|h| < 1 strictly, so
    round(h*127) has max err 1/254 vs a 2e-2 budget).
  - the batch is split into NSLICES slices; each slice's 8 per-core
    shards upload as 8 parallel streams, device execution and int8
    downloads of earlier slices overlap later uploads.
  - the jitted dispatch function is cached across kernel() calls, and
    donated output buffers are recycled device-side so no zero-buffers
    cross the tunnel in steady state; fetch workers only do the
    GIL-released RPC, dequantization runs on the main thread.

Device kernel (per core, per slice of BS sequences):
  x dram [BS, D, T] f16  --DMA-->  x_raw SBUF [D, BS, T] (t-contiguous
  1KB+ lines)  --DVE shuffle-->  x_sb [D, T, BS] t-major compute layout
  hh SBUF [D, T+1, BS] f16 h history, written per step
  PSUM gx: [D, 4, WS, BS] f32 window (q: 0=z 1=r 2=xn 3=ghn): 3 bulk W
  matmuls per window + 3 recurrent U matmuls per step (q0/q1 accumulate
  onto xz/xr, q3 fresh); per step: 3 MM + 2 ACT (sigmoid zr reads PSUM
  directly, tanh n) + 5 DVE ops; finally hh --DVE shuffle--> y_stg
  [D, BS, T] int8 --DMA--> y dram [BS, D, T].
PSUM discipline: exactly one start=True matmul per PSUM tile (the first
bulk gx matmul; clears the bank's has_written bits); every other matmul
uses start=False (accumulates where written, overwrites fresh regions).
All matmul output APs are contiguous (strided PSUM outs crash the
device). A DVE op may read at most ONE operand from PSUM (NCC_IBVF027).
"""

import sys
import numpy as np
from collections import deque
from contextlib import ExitStack
from concurrent.futures import ThreadPoolExecutor

sys.path.insert(0, "/opt/trn_rl_repo")

B_TOT, D, T = 256, 128, 2048
NCORES = 8
B_CORE = B_TOT // NCORES          # 32 sequences per core

# --- tunables ---------------------------------------------------------
NSLICES = 4                       # pipeline depth over the batch axis
BS = B_CORE // NSLICES            # sequences per core per slice
WS = 16                           # recurrence steps per PSUM gx window
X_DT = "f16"                      # tunnel dtype for x: "f16" | "f8"
Y_DT = "i8"                       # tunnel dtype for y: "f16" | "i8"
Y_SCALE = 127.0                   # int8 output scale (|h| < 1 always)
DL_THREADS = 8                    # download worker threads
UL_THREADS = 8                    # upload worker threads

_prog_cache = {}


def _np_dt(name):
    import ml_dtypes
    return {"f16": np.float16, "f8": ml_dtypes.float8_e4m3fn,
            "i8": np.int8}[name]


def _build(b_nonzero: bool):
    import concourse.bacc as bacc
    import concourse.tile as tile
    import concourse.mybir as mybir

    F32 = mybir.dt.float32
    F16 = mybir.dt.float16
    XDT = {"f16": F16, "f8": mybir.dt.float8e4}[X_DT]
    YDT = {"f16": F16, "i8": mybir.dt.int8}[Y_DT]
    SIG = mybir.ActivationFunctionType.Sigmoid
    TANH = mybir.ActivationFunctionType.Tanh

    NW = T // WS                  # gx windows over the full sequence

    nc = bacc.Bacc("TRN2", target_bir_lowering=False, debug=False,
                   num_devices=NCORES)
    x_d = nc.declare_dram_parameter("x", [BS, D, T], XDT, isOutput=False)
    y_d = nc.declare_dram_parameter("y", [BS, D, T], YDT, isOutput=True)
    w_d = nc.declare_dram_parameter("w", [D, 3 * D], F16, isOutput=False)
    u_d = nc.declare_dram_parameter("u", [D, 3 * D], F16, isOutput=False)
    if b_nonzero:
        b_d = nc.declare_dram_parameter("b", [3, D], F32, isOutput=False)

    with tile.TileContext(nc) as tc:
        with ExitStack() as ctx:
            wpool = ctx.enter_context(tc.tile_pool(name="wts", bufs=1))
            xpool = ctx.enter_context(tc.tile_pool(name="xin", bufs=1))
            hpool = ctx.enter_context(tc.tile_pool(name="hh", bufs=1))
            spool = ctx.enter_context(tc.tile_pool(name="small", bufs=3))
            gxpool = ctx.enter_context(
                tc.tile_pool(name="psgx", bufs=2, space="PSUM"))

            wz = wpool.tile([D, D], F16, name="wz")
            wr = wpool.tile([D, D], F16, name="wr")
            wn = wpool.tile([D, D], F16, name="wn")
            uz = wpool.tile([D, D], F16, name="uz")
            ur = wpool.tile([D, D], F16, name="ur")
            un = wpool.tile([D, D], F16, name="un")
            for j, (wt, ut) in enumerate([(wz, uz), (wr, ur), (wn, un)]):
                nc.sync.dma_start(wt[:], w_d[:, j * D:(j + 1) * D])
                nc.sync.dma_start(ut[:], u_d[:, j * D:(j + 1) * D])
            if b_nonzero:
                bz = wpool.tile([D, 1], F32, name="bz")
                br = wpool.tile([D, 1], F32, name="br")
                bn = wpool.tile([D, 1], F32, name="bn")
                for j, bt in enumerate([bz, br, bn]):
                    nc.sync.dma_start(bt[:], b_d[j:j + 1, :].transpose([1, 0]))

            # x arrives batch-major (cheap DMA: contiguous t-lines), then
            # one DVE shuffle to the t-major compute layout [D, T, BS].
            x_raw = xpool.tile([D, BS, T], XDT, name="xraw")
            nc.sync.dma_start(x_raw[:], x_d[:].transpose([1, 0, 2]))
            x_sb = xpool.tile([D, T, BS], F16, name="x")
            nc.vector.tensor_copy(x_sb[:], x_raw[:].transpose([0, 2, 1]))

            hh = hpool.tile([D, T + 1, BS], F16, name="h")
            nc.vector.memset(hh[:, 0:1, :], 0.0)

            for w in range(NW):
                # window PSUM tile: q slots 0=z 1=r 2=xn 3=ghn. One
                # start=True per tile (first bulk matmul) clears the
                # bank; recurrent matmuls accumulate (q0/q1) or fill a
                # fresh region (q3). All matmul out APs contiguous.
                gx = gxpool.tile([D, 4, WS, BS], F32, tag="gx",
                                 name=f"gx{w}")
                xg = x_sb[:, w * WS:(w + 1) * WS, :]
                nc.tensor.matmul(gx[:, 0:1, :, :], wz[:], xg,
                                 start=True, stop=True,
                                 skip_group_check=True)
                nc.tensor.matmul(gx[:, 1:2, :, :], wr[:], xg,
                                 start=False, stop=True,
                                 skip_group_check=True)
                nc.tensor.matmul(gx[:, 2:3, :, :], wn[:], xg,
                                 start=False, stop=True,
                                 skip_group_check=True)

                for tl in range(WS):
                    t = w * WS + tl
                    h_at = hh[:, t:t + 1, :]
                    nc.tensor.matmul(gx[:, 0:1, tl:tl + 1, :], uz[:],
                                     h_at, start=False, stop=True,
                                     skip_group_check=True)
                    nc.tensor.matmul(gx[:, 1:2, tl:tl + 1, :], ur[:],
                                     h_at, start=False, stop=True,
                                     skip_group_check=True)
                    nc.tensor.matmul(gx[:, 3:4, tl:tl + 1, :], un[:],
                                     h_at, start=False, stop=True,
                                     skip_group_check=True)

                    zr = spool.tile([D, 2, BS], F32, tag="zr",
                                    name=f"zr{t}")
                    if b_nonzero:
                        nc.scalar.activation(zr[:, 0:1, :],
                                             gx[:, 0:1, tl:tl + 1, :],
                                             SIG, bias=bz[:])
                        nc.scalar.activation(zr[:, 1:2, :],
                                             gx[:, 1:2, tl:tl + 1, :],
                                             SIG, bias=br[:])
                    else:
                        nc.scalar.activation(zr[:],
                                             gx[:, 0:2, tl:tl + 1, :],
                                             SIG)

                    t1 = spool.tile([D, BS], F32, tag="t1", name=f"t1{t}")
                    nc.vector.tensor_mul(t1[:], zr[:, 1:2, :],
                                         gx[:, 3:4, tl:tl + 1, :])
                    t2 = spool.tile([D, BS], F32, tag="t2", name=f"t2{t}")
                    nc.vector.tensor_add(t2[:], t1[:],
                                         gx[:, 2:3, tl:tl + 1, :])
                    nt = spool.tile([D, BS], F32, tag="nt", name=f"nt{t}")
                    if b_nonzero:
                        nc.scalar.activation(nt[:], t2[:], TANH, bias=bn[:])
                    else:
                        nc.scalar.activation(nt[:], t2[:], TANH)

                    dd = spool.tile([D, BS], F32, tag="dd", name=f"dd{t}")
                    nc.vector.tensor_sub(dd[:], h_at, nt[:])
                    ee = spool.tile([D, BS], F32, tag="ee", name=f"ee{t}")
                    nc.vector.tensor_mul(ee[:], zr[:, 0:1, :], dd[:])
                    nc.vector.tensor_add(hh[:, t + 1:t + 2, :], ee[:],
                                         nt[:])

            # shuffle back to batch-major and DMA out (contiguous t-lines)
            y_stg = hpool.tile([D, BS, T], YDT, name="ystg")
            if Y_DT == "i8":
                nc.vector.tensor_scalar_mul(
                    y_stg[:], hh[:, 1:T + 1, :].transpose([0, 2, 1]),
                    Y_SCALE)
            else:
                nc.vector.tensor_copy(
                    y_stg[:], hh[:, 1:T + 1, :].transpose([0, 2, 1]))
            nc.sync.dma_start(y_d[:].transpose([1, 0, 2]), y_stg[:])
    nc.compile()
    return nc


class _Runner:
    """Cached jitted SPMD dispatch for one compiled bass program.

    Mirrors concourse.bass_utils.run_bass_kernel_spmd's axon branch
    (bass2jax._bass_exec_p under jit+shard_map on the 8 NeuronCores) but
    keeps the traced function and donated output buffers alive across
    calls so steady-state calls move only x up and y down the tunnel.
    """

    def __init__(self, nc):
        import jax
        import concourse.mybir as mybir
        from concourse import bass2jax
        from jax.sharding import Mesh, PartitionSpec, NamedSharding
        from jax.experimental.shard_map import shard_map

        bass2jax.install_neuronx_cc_hook()
        self.nc = nc
        part_name = (nc.partition_id_tensor.name
                     if nc.partition_id_tensor else None)
        in_names, out_names, out_avals = [], [], []
        for alloc in nc.m.functions[0].allocations:
            if not isinstance(alloc, mybir.MemoryLocationSet):
                continue
            name = alloc.memorylocations[0].name
            if alloc.kind == "ExternalInput":
                if name != part_name:
                    in_names.append(name)
            elif alloc.kind == "ExternalOutput":
                out_names.append(name)
                out_avals.append(jax.core.ShapedArray(
                    tuple(alloc.tensor_shape), mybir.dt.np(alloc.dtype)))
        self.in_names = in_names
        self.out_names = out_names
        all_names = in_names + out_names
        n_args = len(all_names)
        if part_name is not None:
            all_names = all_names + [part_name]
        all_names = tuple(all_names)

        def _body(*args):
            operands = list(args)
            if part_name is not None:
                operands.append(bass2jax.partition_id_tensor())
            outs = bass2jax._bass_exec_p.bind(
                *operands,
                out_avals=tuple(out_avals),
                in_names=all_names,
                out_names=tuple(out_names),
                lowering_input_output_aliases=(),
                sim_require_finite=True,
                sim_require_nnan=True,
                nc=nc,
            )
            return tuple(outs)

        devices = jax.devices()[:NCORES]
        self.mesh = Mesh(np.asarray(devices), ("core",))
        P = PartitionSpec
        self.sharding = NamedSharding(self.mesh, P("core"))
        donate = tuple(range(len(in_names), n_args))
        self.jitted = jax.jit(
            shard_map(_body, mesh=self.mesh, in_specs=(P("core"),) * n_args,
                      out_specs=(P("core"),) * len(out_names),
                      check_rep=False),
            donate_argnums=donate, keep_unused=True)
        self.donors = deque()     # device buffers reusable as donated outs
        self.out_shape = tuple(out_avals[0].shape)
        self.out_np_dtype = np.dtype(out_avals[0].dtype)

    def get_donor(self):
        import jax
        if self.donors:
            return self.donors.popleft()
        z = np.zeros((NCORES * self.out_shape[0],) + self.out_shape[1:],
                     self.out_np_dtype)
        return jax.device_put(z, self.sharding)


def _get_nc(b_nonzero: bool):
    key = ("nc", b_nonzero, NSLICES, X_DT, Y_DT)
    if key not in _prog_cache:
        _prog_cache[key] = _build(b_nonzero)
    return _prog_cache[key]


def _get_runner(b_nonzero: bool) -> "_Runner":
    key = ("run", b_nonzero, NSLICES, X_DT, Y_DT)
    if key not in _prog_cache:
        _prog_cache[key] = _Runner(_get_nc(b_nonzero))
    return _prog_cache[key]


def _kernel_fast(x, W, U, b, b_nonzero):
    import jax

    runner = _get_runner(b_nonzero)
    xdt = _np_dt(X_DT)

    wg = jax.device_put(np.tile(W.astype(np.float16), (NCORES, 1)),
                        runner.sharding)
    ug = jax.device_put(np.tile(U.astype(np.float16), (NCORES, 1)),
                        runner.sharding)
    extra = [wg, ug]
    if b_nonzero:
        bg = jax.device_put(
            np.tile(np.ascontiguousarray(b.reshape(3, D)), (NCORES, 1)),
            runner.sharding)
        extra.append(bg)

    y = np.empty((B_TOT, D, T), dtype=np.float32)
    inv_scale = np.float32(1.0 / Y_SCALE)

    def fetch_shard(data):
        # RPC fetch only (releases the GIL); dequant happens on the
        # main thread to avoid starving the upload casts
        return np.asarray(data)

    def store_shard(shard, core, sl):
        r0 = core * B_CORE + sl * BS
        if Y_DT == "i8":
            np.multiply(shard, inv_scale, out=y[r0:r0 + BS])
        else:
            y[r0:r0 + BS] = shard

    import os
    import time
    dbg = bool(os.environ.get("KERNEL_DEBUG_TIMING"))
    t_start = time.time()

    devices = runner.mesh.devices.flat
    glob_shape = (NCORES * BS, D, T)

    def put_shard(arr, dev):
        return jax.device_put(arr, dev)

    pending = []
    with ThreadPoolExecutor(DL_THREADS) as pool, \
            ThreadPoolExecutor(UL_THREADS) as upool:
        for sl in range(NSLICES):
            t0 = time.time()
            # cast per-core shards on the main thread (1-CPU host:
            # worker-side casts just thrash the GIL), then upload as 8
            # parallel streams (the tunnel is stream-limited: ~20MB/s
            # per stream, ~50MB/s aggregate with >=4 streams)
            shards = [
                x[i * B_CORE + sl * BS: i * B_CORE + (sl + 1) * BS]
                .astype(xdt) for i in range(NCORES)
            ]
            put_futs = [upool.submit(put_shard, shards[i], devices[i])
                        for i in range(NCORES)]
            x_dev = jax.make_array_from_single_device_arrays(
                glob_shape, runner.sharding,
                [f.result() for f in put_futs])
            t1 = time.time()
            donor = runner.get_donor()
            (y_dev,) = runner.jitted(x_dev, *extra, donor)
            t2 = time.time()
            # pre-issue host copies so fetch RPCs pipeline instead of
            # paying a round-trip latency per shard
            datas = [s.data for s in y_dev.addressable_shards]
            for d_ in datas:
                d_.copy_to_host_async()
            futs = [pool.submit(fetch_shard, datas[i])
                    for i in range(NCORES)]
            pending.append((y_dev, futs))
            if dbg:
                print(f"  sl{sl}: cast {t1-t0:.2f} put+disp {t2-t1:.2f} "
                      f"(t={t2-t_start:.2f})", flush=True)
        for sl, (y_dev, futs) in enumerate(pending):
            for i, f in enumerate(futs):
                store_shard(f.result(), i, sl)
            if dbg:
                print(f"  sl{sl} stored at t={time.time()-t_start:.2f}",
                      flush=True)
            runner.donors.append(y_dev)
    if dbg:
        print(f"  kernel_fast total {time.time()-t_start:.2f}", flush=True)
    return y


def _kernel_fallback(x, W, U, b, b_nonzero):
    """Library-path fallback: same program via run_bass_kernel_spmd."""
    from concourse.bass_utils import run_bass_kernel_spmd

    nc = _get_nc(b_nonzero)
    xdt = _np_dt(X_DT)
    wg = {"w": W.astype(np.float16), "u": U.astype(np.float16)}
    if b_nonzero:
        wg["b"] = np.ascontiguousarray(b.reshape(3, D))
    y = np.empty((B_TOT, D, T), dtype=np.float32)
    for sl in range(NSLICES):
        in_maps = []
        for i in range(NCORES):
            src = x[i * B_CORE + sl * BS: i * B_CORE + (sl + 1) * BS]
            m = {"x": np.ascontiguousarray(src).astype(xdt)}
            m.update(wg)
            in_maps.append(m)
        res = run_bass_kernel_spmd(nc, in_maps, list(range(NCORES)))
        for i in range(NCORES):
            yi = res.results[i]["y"]
            r0 = i * B_CORE + sl * BS
            if Y_DT == "i8":
                y[r0:r0 + BS] = yi.astype(np.float32) * (1.0 / Y_SCALE)
            else:
                y[r0:r0 + BS] = yi
    return y


def kernel(x, W, U, b):
    x = np.asarray(x, dtype=np.float32)
    W = np.asarray(W, dtype=np.float32)
    U = np.asarray(U, dtype=np.float32)
    b = np.asarray(b, dtype=np.float32)
    b_nonzero = bool(np.any(b != 0.0))
    try:
        return _kernel_fast(x, W, U, b, b_nonzero)
    except Exception:
        import traceback
        traceback.print_exc()
        return _kernel_fallback(x, W, U, b, b_nonzero)


# revision 20
# speedup vs baseline: 1.5184x; 1.0171x over previous
"""GRU layer kernel for Trainium2 (8 NeuronCores, batch-data-parallel).

x: [256, 128, 2048] f32, W/U: [128, 384], b: [384] -> y: [256, 128, 2048] f32

The end-to-end wall time of kernel() is dominated by the axon tunnel
(~20MB/s per stream, ~50MB/s aggregate with >=4 concurrent streams)
and the 1-CPU client host, so the design minimizes and pipelines
host<->device traffic:
  - x crosses the tunnel as f16 (fp8 fails numerically: the gx noise
    drives an h random walk, rel err 0.17) in its NATURAL layout (no
    host-side transposes); y returns as int8 (|h| < 1 strictly, so
    round(h*127) has max err 1/254 vs a 2e-2 budget).
  - the batch is split into NSLICES slices; each slice's 8 per-core
    shards upload as 8 parallel streams, device execution and int8
    downloads of earlier slices overlap later uploads.
  - the jitted dispatch function is cached across kernel() calls, and
    donated output buffers are recycled device-side so no zero-buffers
    cross the tunnel in steady state; fetch workers only do the
    GIL-released RPC, dequantization runs on the main thread.

Device kernel (per core, per slice of BS sequences):
  x dram [BS, D, T] f16  --DMA-->  x_raw SBUF [D, BS, T] (t-contiguous
  1KB+ lines)  --DVE shuffle-->  x_sb [D, T, BS] t-major compute layout
  hh SBUF [D, T+1, BS] f16 h history, written per step
  PSUM gx: [D, 4, WS, BS] f32 window (q: 0=z 1=r 2=xn 3=ghn): 3 bulk W
  matmuls per window + 3 recurrent U matmuls per step (q0/q1 accumulate
  onto xz/xr, q3 fresh); per step: 3 MM + 2 ACT (sigmoid zr reads PSUM
  directly, tanh n) + 5 DVE ops; finally hh --DVE shuffle--> y_stg
  [D, BS, T] int8 --DMA--> y dram [BS, D, T].
PSUM discipline: exactly one start=True matmul per PSUM tile (the first
bulk gx matmul; clears the bank's has_written bits); every other matmul
uses start=False (accumulates where written, overwrites fresh regions).
All matmul output APs are contiguous (strided PSUM outs crash the
device). A DVE op may read at most ONE operand from PSUM (NCC_IBVF027).
"""

import sys
import numpy as np
from collections import deque
from contextlib import ExitStack
from concurrent.futures import ThreadPoolExecutor

sys.path.insert(0, "/opt/trn_rl_repo")

B_TOT, D, T = 256, 128, 2048
NCORES = 8
B_CORE = B_TOT // NCORES          # 32 sequences per core

# --- tunables ---------------------------------------------------------
NSLICES = 4                       # pipeline depth over the batch axis
BS = B_CORE // NSLICES            # sequences per core per slice
WS = 16                           # recurrence steps per PSUM gx window
X_DT = "f16"                      # tunnel dtype for x: "f16" | "f8"
Y_DT = "i8"                       # tunnel dtype for y: "f16" | "i8"
Y_SCALE = 127.0                   # int8 output scale (|h| < 1 always)
DL_THREADS = 4                    # download worker threads
UL_THREADS = 4                    # upload worker threads

_prog_cache = {}


def _np_dt(name):
    import ml_dtypes
    return {"f16": np.float16, "f8": ml_dtypes.float8_e4m3fn,
            "i8": np.int8}[name]


def _build(b_nonzero: bool):
    import concourse.bacc as bacc
    import concourse.tile as tile
    import concourse.mybir as mybir

    F32 = mybir.dt.float32
    F16 = mybir.dt.float16
    XDT = {"f16": F16, "f8": mybir.dt.float8e4}[X_DT]
    YDT = {"f16": F16, "i8": mybir.dt.int8}[Y_DT]
    SIG = mybir.ActivationFunctionType.Sigmoid
    TANH = mybir.ActivationFunctionType.Tanh

    NW = T // WS                  # gx windows over the full sequence

    nc = bacc.Bacc("TRN2", target_bir_lowering=False, debug=False,
                   num_devices=NCORES)
    x_d = nc.declare_dram_parameter("x", [BS, D, T], XDT, isOutput=False)
    y_d = nc.declare_dram_parameter("y", [BS, D, T], YDT, isOutput=True)
    w_d = nc.declare_dram_parameter("w", [D, 3 * D], F16, isOutput=False)
    u_d = nc.declare_dram_parameter("u", [D, 3 * D], F16, isOutput=False)
    if b_nonzero:
        b_d = nc.declare_dram_parameter("b", [3, D], F32, isOutput=False)

    with tile.TileContext(nc) as tc:
        with ExitStack() as ctx:
            wpool = ctx.enter_context(tc.tile_pool(name="wts", bufs=1))
            xpool = ctx.enter_context(tc.tile_pool(name="xin", bufs=1))
            hpool = ctx.enter_context(tc.tile_pool(name="hh", bufs=1))
            spool = ctx.enter_context(tc.tile_pool(name="small", bufs=3))
            gxpool = ctx.enter_context(
                tc.tile_pool(name="psgx", bufs=2, space="PSUM"))

            wz = wpool.tile([D, D], F16, name="wz")
            wr = wpool.tile([D, D], F16, name="wr")
            wn = wpool.tile([D, D], F16, name="wn")
            uz = wpool.tile([D, D], F16, name="uz")
            ur = wpool.tile([D, D], F16, name="ur")
            un = wpool.tile([D, D], F16, name="un")
            for j, (wt, ut) in enumerate([(wz, uz), (wr, ur), (wn, un)]):
                nc.sync.dma_start(wt[:], w_d[:, j * D:(j + 1) * D])
                nc.sync.dma_start(ut[:], u_d[:, j * D:(j + 1) * D])
            if b_nonzero:
                bz = wpool.tile([D, 1], F32, name="bz")
                br = wpool.tile([D, 1], F32, name="br")
                bn = wpool.tile([D, 1], F32, name="bn")
                for j, bt in enumerate([bz, br, bn]):
                    nc.sync.dma_start(bt[:], b_d[j:j + 1, :].transpose([1, 0]))

            # x arrives batch-major (cheap DMA: contiguous t-lines), then
            # one DVE shuffle to the t-major compute layout [D, T, BS].
            x_raw = xpool.tile([D, BS, T], XDT, name="xraw")
            nc.sync.dma_start(x_raw[:], x_d[:].transpose([1, 0, 2]))
            x_sb = xpool.tile([D, T, BS], F16, name="x")
            nc.vector.tensor_copy(x_sb[:], x_raw[:].transpose([0, 2, 1]))

            hh = hpool.tile([D, T + 1, BS], F16, name="h")
            nc.vector.memset(hh[:, 0:1, :], 0.0)

            for w in range(NW):
                # window PSUM tile: q slots 0=z 1=r 2=xn 3=ghn. One
                # start=True per tile (first bulk matmul) clears the
                # bank; recurrent matmuls accumulate (q0/q1) or fill a
                # fresh region (q3). All matmul out APs contiguous.
                gx = gxpool.tile([D, 4, WS, BS], F32, tag="gx",
                                 name=f"gx{w}")
                xg = x_sb[:, w * WS:(w + 1) * WS, :]
                nc.tensor.matmul(gx[:, 0:1, :, :], wz[:], xg,
                                 start=True, stop=True,
                                 skip_group_check=True)
                nc.tensor.matmul(gx[:, 1:2, :, :], wr[:], xg,
                                 start=False, stop=True,
                                 skip_group_check=True)
                nc.tensor.matmul(gx[:, 2:3, :, :], wn[:], xg,
                                 start=False, stop=True,
                                 skip_group_check=True)

                for tl in range(WS):
                    t = w * WS + tl
                    h_at = hh[:, t:t + 1, :]
                    nc.tensor.matmul(gx[:, 0:1, tl:tl + 1, :], uz[:],
                                     h_at, start=False, stop=True,
                                     skip_group_check=True)
                    nc.tensor.matmul(gx[:, 1:2, tl:tl + 1, :], ur[:],
                                     h_at, start=False, stop=True,
                                     skip_group_check=True)
                    nc.tensor.matmul(gx[:, 3:4, tl:tl + 1, :], un[:],
                                     h_at, start=False, stop=True,
                                     skip_group_check=True)

                    zr = spool.tile([D, 2, BS], F32, tag="zr",
                                    name=f"zr{t}")
                    if b_nonzero:
                        nc.scalar.activation(zr[:, 0:1, :],
                                             gx[:, 0:1, tl:tl + 1, :],
                                             SIG, bias=bz[:])
                        nc.scalar.activation(zr[:, 1:2, :],
                                             gx[:, 1:2, tl:tl + 1, :],
                                             SIG, bias=br[:])
                    else:
                        nc.scalar.activation(zr[:],
                                             gx[:, 0:2, tl:tl + 1, :],
                                             SIG)

                    t1 = spool.tile([D, BS], F32, tag="t1", name=f"t1{t}")
                    nc.vector.tensor_mul(t1[:], zr[:, 1:2, :],
                                         gx[:, 3:4, tl:tl + 1, :])
                    t2 = spool.tile([D, BS], F32, tag="t2", name=f"t2{t}")
                    nc.vector.tensor_add(t2[:], t1[:],
                                         gx[:, 2:3, tl:tl + 1, :])
                    nt = spool.tile([D, BS], F32, tag="nt", name=f"nt{t}")
                    if b_nonzero:
                        nc.scalar.activation(nt[:], t2[:], TANH, bias=bn[:])
                    else:
                        nc.scalar.activation(nt[:], t2[:], TANH)

                    dd = spool.tile([D, BS], F32, tag="dd", name=f"dd{t}")
                    nc.vector.tensor_sub(dd[:], h_at, nt[:])
                    ee = spool.tile([D, BS], F32, tag="ee", name=f"ee{t}")
                    nc.vector.tensor_mul(ee[:], zr[:, 0:1, :], dd[:])
                    nc.vector.tensor_add(hh[:, t + 1:t + 2, :], ee[:],
                                         nt[:])

            # shuffle back to batch-major and DMA out (contiguous t-lines)
            y_stg = hpool.tile([D, BS, T], YDT, name="ystg")
            if Y_DT == "i8":
                nc.vector.tensor_scalar_mul(
                    y_stg[:], hh[:, 1:T + 1, :].transpose([0, 2, 1]),
                    Y_SCALE)
            else:
                nc.vector.tensor_copy(
                    y_stg[:], hh[:, 1:T + 1, :].transpose([0, 2, 1]))
            nc.sync.dma_start(y_d[:].transpose([1, 0, 2]), y_stg[:])
    nc.compile()
    return nc


class _Runner:
    """Cached jitted SPMD dispatch for one compiled bass program.

    Mirrors concourse.bass_utils.run_bass_kernel_spmd's axon branch
    (bass2jax._bass_exec_p under jit+shard_map on the 8 NeuronCores) but
    keeps the traced function and donated output buffers alive across
    calls so steady-state calls move only x up and y down the tunnel.
    """

    def __init__(self, nc):
        import jax
        import concourse.mybir as mybir
        from concourse import bass2jax
        from jax.sharding import Mesh, PartitionSpec, NamedSharding
        from jax.experimental.shard_map import shard_map

        bass2jax.install_neuronx_cc_hook()
        self.nc = nc
        part_name = (nc.partition_id_tensor.name
                     if nc.partition_id_tensor else None)
        in_names, out_names, out_avals = [], [], []
        for alloc in nc.m.functions[0].allocations:
            if not isinstance(alloc, mybir.MemoryLocationSet):
                continue
            name = alloc.memorylocations[0].name
            if alloc.kind == "ExternalInput":
                if name != part_name:
                    in_names.append(name)
            elif alloc.kind == "ExternalOutput":
                out_names.append(name)
                out_avals.append(jax.core.ShapedArray(
                    tuple(alloc.tensor_shape), mybir.dt.np(alloc.dtype)))
        self.in_names = in_names
        self.out_names = out_names
        all_names = in_names + out_names
        n_args = len(all_names)
        if part_name is not None:
            all_names = all_names + [part_name]
        all_names = tuple(all_names)

        def _body(*args):
            operands = list(args)
            if part_name is not None:
                operands.append(bass2jax.partition_id_tensor())
            outs = bass2jax._bass_exec_p.bind(
                *operands,
                out_avals=tuple(out_avals),
                in_names=all_names,
                out_names=tuple(out_names),
                lowering_input_output_aliases=(),
                sim_require_finite=True,
                sim_require_nnan=True,
                nc=nc,
            )
            return tuple(outs)

        devices = jax.devices()[:NCORES]
        self.mesh = Mesh(np.asarray(devices), ("core",))
        P = PartitionSpec
        self.sharding = NamedSharding(self.mesh, P("core"))
        donate = tuple(range(len(in_names), n_args))
        self.jitted = jax.jit(
            shard_map(_body, mesh=self.mesh, in_specs=(P("core"),) * n_args,
                      out_specs=(P("core"),) * len(out_names),
                      check_rep=False),
            donate_argnums=donate, keep_unused=True)
        self.donors = deque()     # device buffers reusable as donated outs
        self.out_shape = tuple(out_avals[0].shape)
        self.out_np_dtype = np.dtype(out_avals[0].dtype)

    def get_donor(self):
        import jax
        if self.donors:
            return self.donors.popleft()
        z = np.zeros((NCORES * self.out_shape[0],) + self.out_shape[1:],
                     self.out_np_dtype)
        return jax.device_put(z, self.sharding)


def _get_nc(b_nonzero: bool):
    key = ("nc", b_nonzero, NSLICES, X_DT, Y_DT)
    if key not in _prog_cache:
        _prog_cache[key] = _build(b_nonzero)
    return _prog_cache[key]


def _get_runner(b_nonzero: bool) -> "_Runner":
    key = ("run", b_nonzero, NSLICES, X_DT, Y_DT)
    if key not in _prog_cache:
        _prog_cache[key] = _Runner(_get_nc(b_nonzero))
    return _prog_cache[key]


def _kernel_fast(x, W, U, b, b_nonzero):
    import jax

    runner = _get_runner(b_nonzero)
    xdt = _np_dt(X_DT)

    wg = jax.device_put(np.tile(W.astype(np.float16), (NCORES, 1)),
                        runner.sharding)
    ug = jax.device_put(np.tile(U.astype(np.float16), (NCORES, 1)),
                        runner.sharding)
    extra = [wg, ug]
    if b_nonzero:
        bg = jax.device_put(
            np.tile(np.ascontiguousarray(b.reshape(3, D)), (NCORES, 1)),
            runner.sharding)
        extra.append(bg)

    y = np.empty((B_TOT, D, T), dtype=np.float32)
    inv_scale = np.float32(1.0 / Y_SCALE)

    def fetch_shard(data):
        # RPC fetch only (releases the GIL); dequant happens on the
        # main thread to avoid starving the upload casts
        return np.asarray(data)

    def store_shard(shard, core, sl):
        r0 = core * B_CORE + sl * BS
        if Y_DT == "i8":
            np.multiply(shard, inv_scale, out=y[r0:r0 + BS])
        else:
            y[r0:r0 + BS] = shard

    import os
    import time
    dbg = bool(os.environ.get("KERNEL_DEBUG_TIMING"))
    t_start = time.time()

    devices = runner.mesh.devices.flat
    glob_shape = (NCORES * BS, D, T)

    def put_shard(arr, dev):
        return jax.device_put(arr, dev)

    pending = []
    with ThreadPoolExecutor(DL_THREADS) as pool, \
            ThreadPoolExecutor(UL_THREADS) as upool:
        for sl in range(NSLICES):
            t0 = time.time()
            # cast per-core shards on the main thread (1-CPU host:
            # worker-side casts just thrash the GIL), then upload as 8
            # parallel streams (the tunnel is stream-limited: ~20MB/s
            # per stream, ~50MB/s aggregate with >=4 streams)
            shards = [
                x[i * B_CORE + sl * BS: i * B_CORE + (sl + 1) * BS]
                .astype(xdt) for i in range(NCORES)
            ]
            put_futs = [upool.submit(put_shard, shards[i], devices[i])
                        for i in range(NCORES)]
            x_dev = jax.make_array_from_single_device_arrays(
                glob_shape, runner.sharding,
                [f.result() for f in put_futs])
            t1 = time.time()
            donor = runner.get_donor()
            (y_dev,) = runner.jitted(x_dev, *extra, donor)
            t2 = time.time()
            # pre-issue host copies so fetch RPCs pipeline instead of
            # paying a round-trip latency per shard
            datas = [s.data for s in y_dev.addressable_shards]
            for d_ in datas:
                d_.copy_to_host_async()
            futs = [pool.submit(fetch_shard, datas[i])
                    for i in range(NCORES)]
            pending.append((y_dev, futs))
            if dbg:
                print(f"  sl{sl}: cast {t1-t0:.2f} put+disp {t2-t1:.2f} "
                      f"(t={t2-t_start:.2f})", flush=True)
        for sl, (y_dev, futs) in enumerate(pending):
            for i, f in enumerate(futs):
                store_shard(f.result(), i, sl)
            if dbg:
                print(f"  sl{sl} stored at t={time.time()-t_start:.2f}",
                      flush=True)
            runner.donors.append(y_dev)
    if dbg:
        print(f"  kernel_fast total {time.time()-t_start:.2f}", flush=True)
    return y


def _kernel_fallback(x, W, U, b, b_nonzero):
    """Library-path fallback: same program via run_bass_kernel_spmd."""
    from concourse.bass_utils import run_bass_kernel_spmd

    nc = _get_nc(b_nonzero)
    xdt = _np_dt(X_DT)
    wg = {"w": W.astype(np.float16), "u": U.astype(np.float16)}
    if b_nonzero:
        wg["b"] = np.ascontiguousarray(b.reshape(3, D))
    y = np.empty((B_TOT, D, T), dtype=np.float32)
    for sl in range(NSLICES):
        in_maps = []
        for i in range(NCORES):
            src = x[i * B_CORE + sl * BS: i * B_CORE + (sl + 1) * BS]
            m = {"x": np.ascontiguousarray(src).astype(xdt)}
            m.update(wg)
            in_maps.append(m)
        res = run_bass_kernel_spmd(nc, in_maps, list(range(NCORES)))
        for i in range(NCORES):
            yi = res.results[i]["y"]
            r0 = i * B_CORE + sl * BS
            if Y_DT == "i8":
                y[r0:r0 + BS] = yi.astype(np.float32) * (1.0 / Y_SCALE)
            else:
                y[r0:r0 + BS] = yi
    return y


def kernel(x, W, U, b):
    x = np.asarray(x, dtype=np.float32)
    W = np.asarray(W, dtype=np.float32)
    U = np.asarray(U, dtype=np.float32)
    b = np.asarray(b, dtype=np.float32)
    b_nonzero = bool(np.any(b != 0.0))
    try:
        return _kernel_fast(x, W, U, b, b_nonzero)
    except Exception:
        import traceback
        traceback.print_exc()
        return _kernel_fallback(x, W, U, b, b_nonzero)
